# revision 1
# baseline (speedup 1.0000x reference)
"""TransE-style edge scoring on 8 Trainium2 NeuronCores.

out[e] = sum_d | h[row[e], d] + g[type[e], d] - h[col[e], d] |

Strategy
--------
Edges are data-parallel across the 8 cores. dma_gather (the SWDGE
embedding-row gather) takes int16 indices, so node ids (< 50000) don't
fit directly: edges are bucketed by quadrant (row < 25000, col < 25000)
and each quadrant is split across 2 cores. Every core receives as
*input data* the 25000-row slice of the node table its row-indices and
col-indices refer to, so one SPMD program serves all 8 cores.

Per core, edges are processed in chunks of C: three dma_gathers pull
h[row], h[col], g[type] rows into SBUF with partition = edge-in-chunk
(i%128) layout, then DVE computes (hr - hc) + gt and a fused
abs+reduce along the feature axis produces the per-edge score.

Tables are converted to fp16 on the host: halves the gather traffic
(256B rows) at ~1e-4 relative error on the final sums, far inside f32
output tolerances for this op. Scores accumulate in f32.
"""

import os
import sys

sys.path.insert(0, "/opt/trn_rl_repo")

import numpy as np

import concourse.bass as bass
import concourse.tile as tile
from concourse import bacc, mybir
from concourse.bass_utils import run_bass_kernel_spmd

N_NODES = 50000
HALF = 25000
N_REL = 500
D = 128
N_EDGES = 600000
NCORES = 8

C = 1024              # edges per chunk (dma_gather num_idxs ucode limit)
NB = C // 128         # gathered blocks per chunk
SEG = C // 16         # idx elements per partition (16-way wrap)
GB = 8                # chunks per output store group

_USE_F32 = os.environ.get("KB_F32", "0") == "1"
if _USE_F32:
    TAB_DT, TAB_NP = mybir.dt.float32, np.float32
else:
    TAB_DT, TAB_NP = mybir.dt.float16, np.float16

_programs: dict[int, "bass.Bass"] = {}


def _build_program(nch: int) -> "bass.Bass":
    nc = bacc.Bacc("TRN2", debug=False, dynamic_dma_scratch_size=32768)
    ng = -(-nch // GB)  # output store groups
    hrow = nc.declare_dram_parameter("hrow", [HALF, D], TAB_DT, isOutput=False)
    hcol = nc.declare_dram_parameter("hcol", [HALF, D], TAB_DT, isOutput=False)
    gtab = nc.declare_dram_parameter("gtab", [N_REL, D], TAB_DT, isOutput=False)
    # combined idx: [:, :, 0:SEG]=row  [:, :, SEG:2SEG]=col  [:, :, 2SEG:]=type
    idx = nc.declare_dram_parameter("idx", [nch, 128, 3 * SEG],
                                    mybir.dt.int16, isOutput=False)
    out = nc.declare_dram_parameter("out", [ng, 128, GB * NB],
                                    mybir.dt.float32, isOutput=True)

    with tile.TileContext(nc) as tc:
        with tc.tile_pool(name="idx", bufs=4) as ipool, \
             tc.tile_pool(name="gat", bufs=3) as gpool, \
             tc.tile_pool(name="tmp", bufs=2) as tpool, \
             tc.tile_pool(name="res", bufs=2) as opool:
            for grp in range(ng):
                sc = opool.tile([128, GB * NB], mybir.dt.float32, tag="sc")
                for j in range(min(GB, nch - grp * GB)):
                    k = grp * GB + j
                    it = ipool.tile([128, 3 * SEG], mybir.dt.int16, tag="it")
                    nc.sync.dma_start(it[:], idx[k])

                    hr = gpool.tile([128, NB, D], TAB_DT, tag="hr")
                    hc = gpool.tile([128, NB, D], TAB_DT, tag="hc")
                    gt = gpool.tile([128, NB, D], TAB_DT, tag="gt")
                    nc.gpsimd.dma_gather(hr[:], hrow[:], it[:, 0:SEG],
                                         num_idxs=C, num_idxs_reg=C,
                                         elem_size=D)
                    nc.gpsimd.dma_gather(hc[:], hcol[:], it[:, SEG:2 * SEG],
                                         num_idxs=C, num_idxs_reg=C,
                                         elem_size=D)
                    nc.gpsimd.dma_gather(gt[:], gtab[:], it[:, 2 * SEG:],
                                         num_idxs=C, num_idxs_reg=C,
                                         elem_size=D)

                    t = tpool.tile([128, NB, D], TAB_DT, tag="t")
                    nc.vector.tensor_tensor(t[:], hr[:], hc[:],
                                            mybir.AluOpType.subtract)
                    t2 = tpool.tile([128, NB, D], TAB_DT, tag="t2")
                    nc.vector.tensor_tensor(t2[:], t[:], gt[:],
                                            mybir.AluOpType.add)
                    nc.vector.tensor_reduce(sc[:, j * NB:(j + 1) * NB], t2[:],
                                            axis=mybir.AxisListType.X,
                                            op=mybir.AluOpType.add,
                                            apply_absolute_value=True)
                nc.sync.dma_start(out[grp], sc[:])
    nc.compile()
    return nc


def _wrap_idx(idx: np.ndarray, nch: int) -> np.ndarray:
    """[nch*C] int16 -> [nch, 128, SEG]: index i of chunk k lives at
    [k, i % 16, i // 16], replicated across the 8 partition groups."""
    w = idx.reshape(nch, SEG, 16).transpose(0, 2, 1)        # [nch, 16, SEG]
    return np.ascontiguousarray(np.tile(w, (1, 8, 1)))       # [nch, 128, SEG]


def _shard(row, col, typ):
    """Assign edges to cores by (row-half, col-half) quadrant, two cores
    per quadrant. Returns (per-core edge-id lists, nch)."""
    quad = (row >= HALF).astype(np.int8) * 2 + (col >= HALF).astype(np.int8)
    order = np.argsort(quad, kind="stable")
    counts = np.bincount(quad, minlength=4)
    perms = []
    off = 0
    for q in range(4):
        ids = order[off:off + counts[q]]
        off += counts[q]
        half = (len(ids) + 1) // 2
        perms.append(ids[:half])
        perms.append(ids[half:])
    ec_max = max(len(p) for p in perms)
    nch = max(1, -(-ec_max // C))
    return perms, nch


def kernel(h, g, edge_idx, edge_type):
    h = np.asarray(h, dtype=np.float32)
    g = np.asarray(g, dtype=np.float32)
    edge_idx = np.asarray(edge_idx)
    row = edge_idx[0].astype(np.int64)
    col = edge_idx[1].astype(np.int64)
    typ = np.asarray(edge_type).astype(np.int64)

    h16 = h.astype(TAB_NP)
    g16 = np.ascontiguousarray(g.astype(TAB_NP))
    h_lo = np.ascontiguousarray(h16[:HALF])
    h_hi = np.ascontiguousarray(h16[HALF:])

    perms, nch = _shard(row, col, typ)

    in_maps = []
    for ci, ids in enumerate(perms):
        quad = ci // 2
        rbase = HALF if quad >= 2 else 0
        cbase = HALF if quad % 2 == 1 else 0
        ec = len(ids)
        pad = nch * C - ec
        r16 = np.concatenate([row[ids] - rbase, np.zeros(pad, np.int64)])
        c16 = np.concatenate([col[ids] - cbase, np.zeros(pad, np.int64)])
        t16 = np.concatenate([typ[ids], np.zeros(pad, np.int64)])
        in_maps.append({
            "hrow": h_hi if quad >= 2 else h_lo,
            "hcol": h_hi if quad % 2 == 1 else h_lo,
            "gtab": g16,
            "idx": np.concatenate([_wrap_idx(r16.astype(np.int16), nch),
                                   _wrap_idx(c16.astype(np.int16), nch),
                                   _wrap_idx(t16.astype(np.int16), nch)],
                                  axis=2),
        })

    if nch not in _programs:
        _programs[nch] = _build_program(nch)
    nc = _programs[nch]

    results = run_bass_kernel_spmd(nc, in_maps, list(range(NCORES))).results

    scores = np.empty(N_EDGES, dtype=np.float32)
    ng = -(-nch // GB)
    for ci, ids in enumerate(perms):
        res = np.asarray(results[ci]["out"])          # [ng, 128, GB*NB]
        vals = (res.reshape(ng, 128, GB, NB)
                .transpose(0, 2, 3, 1)                # (grp, j, b, p)
                .reshape(-1)[:len(ids)])
        scores[ids] = vals
    return scores

